# revision 4
# baseline (speedup 1.0000x reference)
"""Trainium2 Bass kernel (v8) for nn_BaseModel_63058709840114 (dense_mlp).

Reference model per row (d_in=10, d=12):
    h  = x @ We + be
    n1 = LN(h) * g1 + bn1
    m  = relu(n1 @ W1 + b1) @ W2 + b2
    h2 = h + m
    out = (LN(h2) * gh + bnh) @ Wh + bh

v5 design (HW-legal rebalance of v4; ~683us v3 baseline):
  * 10-block feature-major layout ([120 part = 10 blocks x 12 feats,
    512 cols = 4 x 128 batch rows]); weights folded with the centering
    projection C = I - J/12 on the host.
  * LN scale via ACT Abs_reciprocal_sqrt (r = 1/sqrt(var+eps), exact for
    positive input; in the same act table as identity/relu/square so no
    table switches). zs/h2s multiplies on DVE stt; n2 on Pool.
  * HW constraints honored (verified against neuronxcc): GPSIMD/Pool
    cannot touch PSUM, divide is not an ISA ALU op, DMA cannot read PSUM.
    So all six PSUM evacuations go to ACT (wide Identity) or DVE (stt),
    Pool gets only SBUF->SBUF work (squares, relu, n2 multiply).
  * Per-pair engine budget (HW-measured ns; gpsimd tensor_scalar is
    pathologically slow on HW so Pool only runs tensor_mul):
    ACT {hcsW 1343, r1W 1284, r2W 1284, ofW 1343} 5254 | DVE {zs 2x948,
    reluW-ts 352, h2s 2x948, sq2W-tt 605} 4749 | Pool {sq1W-mul 2175,
    n2W-mul 2175} 4350 | PE 12mm 3720 | SP in+out DMA ~1600.
  * 6-stage modulo software pipeline over supertile PAIRS, emitted
    oldest-stage-first so each engine queue only has backward deps.
    PSUM: pW bufs=2 holds the wide 2-bank tiles (hcW/v1W/v2W), pS bufs=4
    the single-bank ones (zp/mp/up) = 8 banks exactly.
  * Output stays feature-major [120, 1024]/pair; host de-transposes.
  * REPEAT passes inside one NEFF; timing = async-batch slope / REPEAT.
"""

import os
import sys
import numpy as np
import ml_dtypes

sys.path.insert(0, "/opt/trn_rl_repo")

EPS = 1e-5
D_IN, D = 10, 12
G = 10                      # feature blocks per supertile
NCOL = 512                  # free dim of compute tiles
ROWS_ST = 128 * G * 4       # rows per supertile = 5120
PAIR_ST = int(os.environ.get("KV8_PAIR_ST", "52"))  # pairs per loop iteration
BATCH_ST = 2 * PAIR_ST
BATCH_ROWS = ROWS_ST * BATCH_ST
N_CORES = 8
REPEAT = int(os.environ.get("KV8_REPEAT", "4"))

LAST_EXEC_NS = None


def _fold_weights(w):
    f64 = {k: np.asarray(v, dtype=np.float64) for k, v in w.items()}
    C = np.eye(D) - np.ones((D, D)) / D
    A1 = f64["w_embed"] @ C
    c1 = f64["b_embed"] @ C
    W1g = np.diag(f64["g_norm1"]) @ f64["w_fc1"]
    b1f = f64["b_norm1"] @ f64["w_fc1"] + f64["b_fc1"]
    W2C = f64["w_fc2"] @ C
    c2 = f64["b_fc2"] @ C
    Whg = np.diag(f64["g_normh"]) @ f64["w_head"]
    bhf = f64["b_normh"] @ f64["w_head"] + f64["b_head"]
    return dict(A1=A1, c1=c1, A1W1=A1 @ W1g, c1w1=c1 @ W1g, b1f=b1f,
                W2C=W2C, c2=c2, Whg=Whg, bhf=bhf)


def _block_diag(M, nblk):
    k, m = M.shape
    out = np.zeros((nblk * k, nblk * m), dtype=M.dtype)
    for t in range(nblk):
        out[t * k:(t + 1) * k, t * m:(t + 1) * m] = M
    return out


def make_consts(w):
    f = _fold_weights(w)
    bf16 = ml_dtypes.bfloat16
    consts = {}
    consts["a1blk"] = _block_diag(f["A1"].astype(np.float32), G).astype(bf16)
    consts["a1w1blk"] = _block_diag(f["A1W1"].astype(np.float32), G).astype(bf16)
    consts["w2cblk"] = _block_diag(f["W2C"].astype(np.float32), G).astype(bf16)
    consts["whgblk"] = _block_diag(f["Whg"].astype(np.float32), G).astype(bf16)
    vrep = np.zeros((120, 120), dtype=np.float32)
    for t in range(G):
        vrep[t * D:(t + 1) * D, t * D:(t + 1) * D] = 1.0 / D
    consts["vrep"] = vrep.astype(bf16)
    consts["c1v"] = np.tile(f["c1"], G).astype(np.float32).reshape(120, 1)
    consts["c1w1v"] = np.tile(f["c1w1"], G).astype(np.float32).reshape(120, 1)
    consts["b1v"] = np.tile(f["b1f"], G).astype(np.float32).reshape(120, 1)
    consts["c2v"] = np.tile(f["c2"], G).astype(np.float32).reshape(120, 1)
    consts["c12v"] = np.tile(f["c1"] + f["c2"], G).astype(
        np.float32).reshape(120, 1)
    consts["bhv"] = np.tile(f["bhf"], G).astype(np.float32).reshape(120, 1)
    consts["epsv"] = np.full((120, 1), EPS, dtype=np.float32)
    return consts


CONST_SPECS = [
    ("a1blk", (100, 120), "bf16"),
    ("a1w1blk", (100, 120), "bf16"),
    ("w2cblk", (120, 120), "bf16"),
    ("whgblk", (120, 120), "bf16"),
    ("vrep", (120, 120), "bf16"),
    ("c1v", (120, 1), "f32"),
    ("c1w1v", (120, 1), "f32"),
    ("b1v", (120, 1), "f32"),
    ("c2v", (120, 1), "f32"),
    ("c12v", (120, 1), "f32"),
    ("bhv", (120, 1), "f32"),
    ("epsv", (120, 1), "f32"),
]


def build_nc(b_core):
    import concourse.bass as bass
    import concourse.bacc as bacc
    import concourse.mybir as mybir
    import concourse.tile as tile

    dt = mybir.dt
    BF, F32 = dt.bfloat16, dt.float32
    AF = mybir.ActivationFunctionType
    OP = mybir.AluOpType

    assert b_core % BATCH_ROWS == 0
    n_batch = b_core // BATCH_ROWS
    n_st_total = b_core // ROWS_ST
    n_pair_total = n_st_total // 2

    nc = bacc.Bacc("TRN2", target_bir_lowering=False, debug=False)
    x_d = nc.dram_tensor("x", [n_st_total + 4, 100, NCOL], BF,
                         kind="ExternalInput")
    out_d = nc.dram_tensor("out", [n_pair_total, 120, 2 * NCOL], BF,
                           kind="ExternalOutput")
    cd = {}
    for name, shape, ty in CONST_SPECS:
        cd[name] = nc.dram_tensor(name, list(shape),
                                  BF if ty == "bf16" else F32,
                                  kind="ExternalInput")

    xvT = x_d.ap().rearrange("(g s) q c -> q g s c", s=2)

    with tile.TileContext(nc) as tc:
        with (
            tc.tile_pool(name="const", bufs=1) as constp,
            tc.tile_pool(name="xin", bufs=8) as xinp,
            tc.tile_pool(name="pW", bufs=3, space="PSUM") as pW,
            tc.tile_pool(name="pS", bufs=2, space="PSUM") as pS,
            tc.tile_pool(name="hcs", bufs=5) as hcsp,
            tc.tile_pool(name="sq1", bufs=3) as sq1p,
            tc.tile_pool(name="r1", bufs=3) as r1p,
            tc.tile_pool(name="zs", bufs=2) as zsp,
            tc.tile_pool(name="aw", bufs=3) as awp,
            tc.tile_pool(name="h2s", bufs=4) as h2sp,
            tc.tile_pool(name="sq2", bufs=3) as sq2p,
            tc.tile_pool(name="r2", bufs=3) as r2p,
            tc.tile_pool(name="n2", bufs=2) as n2p,
            tc.tile_pool(name="of", bufs=3) as ofp,
        ):
            cs = {}
            for name, shape, ty in CONST_SPECS:
                t = constp.tile(list(shape), BF if ty == "bf16" else F32,
                                tag=name, name=name)
                nc.sync.dma_start(out=t[:], in_=cd[name].ap())
                cs[name] = t
            warm = constp.tile([120, 1], F32, name="warm")
            nc.scalar.activation(warm[:], cs["epsv"][:],
                                 AF.Abs_reciprocal_sqrt)

            def load_chunk(i, h, pre=None):
                g = i * PAIR_ST + h
                xt = pre if pre is not None else xinp.tile(
                    [100, 1024], BF, tag="xin", name="xin")
                nc.gpsimd.dma_start(
                    out=xt[:].rearrange("q (s c) -> q s c", s=2),
                    in_=xvT[:, g])
                return xt

            pre0 = load_chunk(0, 0)
            pre1 = load_chunk(0, 1)

            def emit_batch(i):
                xin = [pre0, pre1]
                st = {}

                def grp_hc_mm(j):
                    xt = xin[j]
                    hcW = pW.tile([120, 2 * NCOL], F32, tag="pW", name="hcW")
                    nc.tensor.matmul(hcW[:, 0:512], cs["a1blk"][:],
                                     xt[:, 0:512], start=True, stop=True,
                                     skip_group_check=True)
                    nc.tensor.matmul(hcW[:, 512:1024], cs["a1blk"][:],
                                     xt[:, 512:1024], start=True, stop=True,
                                     skip_group_check=True)
                    st[j] = dict(hcW=hcW)

                def grp_sq1(j):
                    s = st[j]
                    hcW = s.pop("hcW")
                    sq1W = sq1p.tile([120, 1024], BF, name="sq1W")
                    nc.scalar.activation(sq1W[:], hcW[:], AF.Square,
                                         bias=cs["c1v"][:, 0:1])
                    s["sq1W"] = sq1W

                def grp_v1(j):
                    s = st[j]
                    sq1W = s.pop("sq1W")
                    v1W = pW.tile([120, 2 * NCOL], F32, tag="pW", name="v1W")
                    nc.tensor.matmul(v1W[:, 0:512], cs["vrep"][:],
                                     sq1W[:, 0:512], start=True, stop=True,
                                     skip_group_check=True)
                    nc.tensor.matmul(v1W[:, 512:1024], cs["vrep"][:],
                                     sq1W[:, 512:1024], start=True, stop=True,
                                     skip_group_check=True)
                    s["v1W"] = v1W

                def grp_r1(j):
                    s = st[j]
                    v1W = s.pop("v1W")
                    r1W = r1p.tile([120, 1024], F32, name="r1W")
                    nc.scalar.activation(r1W[:], v1W[:],
                                         AF.Abs_reciprocal_sqrt,
                                         bias=cs["epsv"][:, 0:1])
                    s["r1W"] = r1W

                def grp_zp(j):
                    xt = xin[j]
                    s = st[j]
                    zp0 = pS.tile([120, NCOL], F32, tag="pS", name="zp0")
                    nc.tensor.matmul(zp0[:], cs["a1w1blk"][:], xt[:, 0:512],
                                     start=True, stop=True)
                    zp1 = pS.tile([120, NCOL], F32, tag="pS", name="zp1")
                    nc.tensor.matmul(zp1[:], cs["a1w1blk"][:], xt[:, 512:1024],
                                     start=True, stop=True)
                    s["zp"] = (zp0, zp1)

                def grp_zs(j):
                    s = st[j]
                    zp0, zp1 = s.pop("zp")
                    r1W = s.pop("r1W")
                    zsW = zsp.tile([120, 1024], BF, name="zsW")
                    nc.vector.scalar_tensor_tensor(
                        zsW[:, 0:512], zp0[:], cs["c1w1v"][:, 0:1],
                        r1W[:, 0:512], OP.add, OP.mult)
                    nc.vector.scalar_tensor_tensor(
                        zsW[:, 512:1024], zp1[:], cs["c1w1v"][:, 0:1],
                        r1W[:, 512:1024], OP.add, OP.mult)
                    s["zsW"] = zsW

                def grp_relu(j):
                    s = st[j]
                    zsW = s.pop("zsW")
                    aW = awp.tile([120, 1024], BF, name="aW")
                    nc.vector.tensor_scalar(aW[:], zsW[:], cs["b1v"][:, 0:1],
                                            0.0, OP.add, OP.max)
                    s["aW"] = aW

                def grp_mp(j):
                    xt = xin[j]
                    s = st[j]
                    aW = s.pop("aW")
                    h2pW = pW.tile([120, 2 * NCOL], F32, tag="pW",
                                   name="h2pW")
                    nc.tensor.matmul(h2pW[:, 0:512], cs["a1blk"][:],
                                     xt[:, 0:512], start=True, stop=False,
                                     skip_group_check=True)
                    nc.tensor.matmul(h2pW[:, 512:1024], cs["a1blk"][:],
                                     xt[:, 512:1024], start=True, stop=False,
                                     skip_group_check=True)
                    nc.tensor.matmul(h2pW[:, 0:512], cs["w2cblk"][:],
                                     aW[:, 0:512], start=False, stop=True,
                                     skip_group_check=True)
                    nc.tensor.matmul(h2pW[:, 512:1024], cs["w2cblk"][:],
                                     aW[:, 512:1024], start=False, stop=True,
                                     skip_group_check=True)
                    s["h2pW"] = h2pW

                def grp_h2s(j):
                    s = st[j]
                    h2pW = s.pop("h2pW")
                    h2sW = h2sp.tile([120, 1024], BF, name="h2sW")
                    nc.scalar.activation(h2sW[:], h2pW[:], AF.Identity,
                                         bias=cs["c12v"][:, 0:1])
                    s["h2sW"] = h2sW

                def grp_sq2(j):
                    s = st[j]
                    sq2W = sq2p.tile([120, 1024], BF, name="sq2W")
                    nc.vector.tensor_mul(sq2W[:], s["h2sW"][:], s["h2sW"][:])
                    s["sq2W"] = sq2W

                def grp_v2(j):
                    s = st[j]
                    sq2W = s.pop("sq2W")
                    v2W = pW.tile([120, 2 * NCOL], F32, tag="pW", name="v2W")
                    nc.tensor.matmul(v2W[:, 0:512], cs["vrep"][:],
                                     sq2W[:, 0:512], start=True, stop=True,
                                     skip_group_check=True)
                    nc.tensor.matmul(v2W[:, 512:1024], cs["vrep"][:],
                                     sq2W[:, 512:1024], start=True, stop=True,
                                     skip_group_check=True)
                    s["v2W"] = v2W

                def grp_r2(j):
                    s = st[j]
                    v2W = s.pop("v2W")
                    r2W = r2p.tile([120, 1024], BF, name="r2W")
                    nc.scalar.activation(r2W[:], v2W[:],
                                         AF.Abs_reciprocal_sqrt,
                                         bias=cs["epsv"][:, 0:1])
                    s["r2W"] = r2W

                def grp_n2(j):
                    s = st[j]
                    n2W = n2p.tile([120, 1024], BF, name="n2W")
                    nc.vector.tensor_mul(n2W[:], s.pop("h2sW"),
                                         s.pop("r2W"))
                    s["n2W"] = n2W

                def grp_up(j):
                    s = st[j]
                    n2W = s.pop("n2W")
                    upW = pW.tile([120, 2 * NCOL], F32, tag="pW", name="upW")
                    nc.tensor.matmul(upW[:, 0:512], cs["whgblk"][:],
                                     n2W[:, 0:512], start=True, stop=True,
                                     skip_group_check=True)
                    nc.tensor.matmul(upW[:, 512:1024], cs["whgblk"][:],
                                     n2W[:, 512:1024], start=True, stop=True,
                                     skip_group_check=True)
                    s["upW"] = upW

                def grp_of(j):
                    s = st[j]
                    upW = s.pop("upW")
                    ofW = ofp.tile([120, 1024], BF, name="ofW")
                    nc.vector.tensor_scalar(ofW[:], upW[:],
                                            cs["bhv"][:, 0:1], None, OP.add)
                    nc.sync.dma_start(out=out_d.ap()[i * PAIR_ST + j],
                                      in_=ofW[:])
                    del st[j]

                # modulo schedule, oldest stages first per slot
                # op-level interleave: each engine queue starts the slot
                # with already-ready work (n2W/mp/hc), deferring same-slot
                # dependent ops so no engine head-of-line blocks.
                n_slots = PAIR_ST + 5
                for slot in range(n_slots):
                    h_need = slot + 2
                    if h_need < PAIR_ST:
                        xin.append(load_chunk(i, h_need))
                    if slot == PAIR_ST:
                        load_chunk(i + 1, 0, pre=pre0)
                    if slot == PAIR_ST + 1:
                        load_chunk(i + 1, 1, pre=pre1)
                    sF = slot - 5   # n2/up/of
                    sE = slot - 4   # v2/r2
                    sD = slot - 3   # mp/h2s/sq2
                    sC = slot - 2   # zp/zs/relu
                    sB = slot - 1   # v1/r1
                    sA = slot       # hc/hcs/sq1
                    okF = 0 <= sF
                    okE = 0 <= sE < PAIR_ST
                    okD = 0 <= sD < PAIR_ST
                    okC = 0 <= sC < PAIR_ST
                    okB = 0 <= sB < PAIR_ST
                    okA = sA < PAIR_ST
                    if okF:
                        grp_n2(sF)          # DVE, ready at slot start
                    if okE:
                        grp_v2(sE)          # PE, ready
                    if okC:
                        grp_zp(sC)          # PE, ready
                    if okD:
                        grp_mp(sD)          # PE 4mm (hc recompute + W2C acc)
                    if okE:
                        grp_r2(sE)          # ACT after v2
                    if okC:
                        grp_zs(sC)          # DVE after zp + old r1
                    if okA:
                        grp_hc_mm(sA)       # PE, ready
                    if okD:
                        grp_h2s(sD)         # ACT after h2p mms
                    if okF:
                        grp_up(sF)          # PE after n2W
                    if okC:
                        grp_relu(sC)        # DVE after zs
                    if okA:
                        grp_sq1(sA)         # ACT Square after hc
                    if okD:
                        grp_sq2(sD)         # DVE after h2sW
                    if okF:
                        grp_of(sF)          # DVE after up, DMA
                    if okB:
                        grp_v1(sB)          # PE, old sq1W
                    if okB:
                        grp_r1(sB)          # ACT after v1

            with tc.For_i(0, REPEAT, 1) as _rep:
                with tc.For_i(0, n_batch, 1) as i:
                    emit_batch(i)

    nc.compile()
    return nc


def _shard_and_pad(x, b_core):
    B = x.shape[0]
    per = B // N_CORES
    n_st = b_core // ROWS_ST
    shards = []
    for i in range(N_CORES):
        s = x[i * per:(i + 1) * per]
        if b_core > per:
            s = np.concatenate(
                [s, np.zeros((b_core - per, x.shape[1]), x.dtype)])
        xt = np.ascontiguousarray(
            s.reshape(128, n_st, 4, G, D_IN).transpose(1, 3, 4, 2, 0)
        ).reshape(n_st, 100, 512).astype(ml_dtypes.bfloat16)
        xt = np.concatenate([xt, xt[:4]])
        shards.append(xt)
    return shards, per


def _detranspose_out(out_np, n_st, per):
    # out_np: [n_pair, 120, 1024] bf16 -> rows [b_core, 12] fp32
    o = np.asarray(out_np).reshape(n_st // 2, G, D, 2, 4, 128)
    # axes: (pair, t, j, s, c, p) -> (p, pair, s, c, t, j)
    o = o.transpose(5, 0, 3, 4, 1, 2).reshape(128 * n_st * 4 * G, D)
    return o[:per].astype(np.float32)


def kernel(**inputs):
    x = np.asarray(inputs["x"], dtype=np.float32)
    B = x.shape[0]
    per = B // N_CORES
    b_core = ((per + BATCH_ROWS - 1) // BATCH_ROWS) * BATCH_ROWS
    consts = make_consts(
        {k: np.asarray(v) for k, v in inputs.items() if k != "x"})

    nc = build_nc(b_core)
    shards, per = _shard_and_pad(x, b_core)
    in_maps = []
    for i in range(N_CORES):
        m = {"x": shards[i]}
        for name, shape, ty in CONST_SPECS:
            m[name] = np.ascontiguousarray(
                consts[name].astype(
                    ml_dtypes.bfloat16 if ty == "bf16" else np.float32))
        in_maps.append(m)

    results, exec_ns = _run_pjrt(nc, in_maps)
    global LAST_EXEC_NS
    LAST_EXEC_NS = exec_ns
    n_st = b_core // ROWS_ST
    out = np.concatenate(
        [_detranspose_out(r, n_st, per) for r in results], axis=0)
    return out


def _run_pjrt(nc, in_maps):
    """Run the bass program on 8 cores via PJRT (axon) and time steady-state
    execution with inputs already on device (async batch slope)."""
    import time
    import jax
    import concourse.mybir as mybir
    from jax.sharding import Mesh, PartitionSpec
    from jax.experimental.shard_map import shard_map
    from concourse.bass2jax import (
        install_neuronx_cc_hook, _bass_exec_p, partition_id_tensor)

    install_neuronx_cc_hook()
    n_cores = len(in_maps)
    partition_name = (nc.partition_id_tensor.name
                      if nc.partition_id_tensor else None)

    in_names, out_names, out_avals, zero_outs = [], [], [], []
    for alloc in nc.m.functions[0].allocations:
        if not isinstance(alloc, mybir.MemoryLocationSet):
            continue
        name = alloc.memorylocations[0].name
        if alloc.kind == "ExternalInput":
            if name != partition_name:
                in_names.append(name)
        elif alloc.kind == "ExternalOutput":
            shape = tuple(alloc.tensor_shape)
            dtype = mybir.dt.np(alloc.dtype)
            out_names.append(name)
            out_avals.append(jax.core.ShapedArray(shape, dtype))
            zero_outs.append(np.zeros(shape, dtype))
    n_params = len(in_names)
    n_outs = len(out_avals)
    all_names = in_names + out_names
    if partition_name is not None:
        all_names.append(partition_name)
    donate = tuple(range(n_params, n_params + n_outs))

    def _body(*args):
        operands = list(args)
        if partition_name is not None:
            operands.append(partition_id_tensor())
        outs = _bass_exec_p.bind(
            *operands,
            out_avals=tuple(out_avals),
            in_names=tuple(all_names),
            out_names=tuple(out_names),
            lowering_input_output_aliases=(),
            sim_require_finite=True,
            sim_require_nnan=True,
            nc=nc,
        )
        return tuple(outs)

    devices = jax.devices()[:n_cores]
    mesh = Mesh(np.asarray(devices), ("core",))
    sharded = jax.jit(
        shard_map(_body, mesh=mesh,
                  in_specs=(PartitionSpec("core"),) * (n_params + n_outs),
                  out_specs=(PartitionSpec("core"),) * n_outs,
                  check_rep=False),
        donate_argnums=donate, keep_unused=True,
    )
    concat_in = [
        np.concatenate([np.asarray(in_maps[c][nm]) for c in range(n_cores)],
                       axis=0)
        for nm in in_names
    ]
    concat_zeros = [np.zeros((n_cores * z.shape[0], *z.shape[1:]), z.dtype)
                    for z in zero_outs]

    sh = jax.sharding.NamedSharding(mesh, PartitionSpec("core"))
    dev_in = [jax.device_put(a, sh) for a in concat_in]
    out_arrs = jax.block_until_ready(
        sharded(*dev_in, *[jax.device_put(z, sh) for z in concat_zeros]))
    res_np = [np.asarray(o) for o in out_arrs]

    exec_ns = None
    if int(os.environ.get("KERNEL_TIME", "0")):
        try:
            fn2 = jax.jit(
                shard_map(_body, mesh=mesh,
                          in_specs=(PartitionSpec("core"),) * (n_params + n_outs),
                          out_specs=(PartitionSpec("core"),) * n_outs,
                          check_rep=False),
                keep_unused=True)
            zs_dev = [jax.device_put(z, sh) for z in concat_zeros]
            jax.block_until_ready(fn2(*dev_in, *zs_dev))  # warm
            times = {}
            for n in (4, 10, 16, 22):
                best = None
                for _ in range(4):
                    t0 = time.perf_counter()
                    outs_l = [fn2(*dev_in, *zs_dev) for _ in range(n)]
                    jax.block_until_ready(outs_l)
                    dt_ = time.perf_counter() - t0
                    best = dt_ if best is None else min(best, dt_)
                    del outs_l
                times[n] = best
            print(f"async batch times: {times}")
            ns_ = np.array(sorted(times), dtype=np.float64)
            ts_ = np.array([times[int(n)] for n in ns_])
            slope = float(np.polyfit(ns_, ts_, 1)[0])
            exec_ns = int(slope * 1e9 / REPEAT)
        except Exception as e:
            print(f"timing failed: {e}")

    outs = res_np[out_names.index("out")].reshape(
        n_cores, -1, 120, 1024)
    return [outs[c] for c in range(n_cores)], exec_ns


def reference_np(x64, w):
    C = np.eye(D) - np.ones((D, D)) / D

    def ln(h):
        hc = h @ C
        var = (hc * hc).mean(-1, keepdims=True)
        return hc / np.sqrt(var + EPS)

    h = x64 @ w["w_embed"] + w["b_embed"]
    n = ln(h) * w["g_norm1"] + w["b_norm1"]
    m = np.maximum(n @ w["w_fc1"] + w["b_fc1"], 0.0) @ w["w_fc2"] + w["b_fc2"]
    h = h + m
    h = ln(h) * w["g_normh"] + w["b_normh"]
    return h @ w["w_head"] + w["b_head"]


def _patch_sim_absrsqrt():
    """CoreSim lacks Abs_reciprocal_sqrt; emulate via the Rsqrt path
    (identical for positive inputs). Local dev only."""
    import concourse.bass_interp as bi
    import concourse.mybir as mb
    if getattr(bi.InstructionExecutor, "_absrsqrt_patched", False):
        return
    orig = bi.InstructionExecutor.visit_InstActivation

    def patched(self, instruction, **kw):
        if instruction.func == mb.ActivationFunctionType.Abs_reciprocal_sqrt:
            instruction.func = mb.ActivationFunctionType.Rsqrt
            try:
                return orig(self, instruction, **kw)
            finally:
                instruction.func = \
                    mb.ActivationFunctionType.Abs_reciprocal_sqrt
        return orig(self, instruction, **kw)

    bi.InstructionExecutor.visit_InstActivation = patched
    bi.InstructionExecutor._absrsqrt_patched = True


if __name__ == "__main__":
    import concourse.mybir as mybir  # noqa
    from concourse.bass_interp import CoreSim

    _patch_sim_absrsqrt()
    rng = np.random.default_rng(0)
    n_batch = int(sys.argv[1]) if len(sys.argv) > 1 else 1
    b_core = BATCH_ROWS * n_batch
    w = {
        "w_embed": rng.uniform(-0.3, 0.3, (D_IN, D)).astype(np.float32),
        "b_embed": rng.uniform(-0.3, 0.3, (D,)).astype(np.float32),
        "g_norm1": np.ones(D, np.float32), "b_norm1": np.zeros(D, np.float32),
        "w_fc1": rng.uniform(-0.3, 0.3, (D, D)).astype(np.float32),
        "b_fc1": rng.uniform(-0.3, 0.3, (D,)).astype(np.float32),
        "w_fc2": rng.uniform(-0.3, 0.3, (D, D)).astype(np.float32),
        "b_fc2": rng.uniform(-0.3, 0.3, (D,)).astype(np.float32),
        "g_normh": np.ones(D, np.float32), "b_normh": np.zeros(D, np.float32),
        "w_head": rng.uniform(-0.3, 0.3, (D, D)).astype(np.float32),
        "b_head": rng.uniform(-0.3, 0.3, (D,)).astype(np.float32),
    }
    x = rng.standard_normal((b_core, D_IN)).astype(np.float32)
    consts = make_consts(w)

    nc = build_nc(b_core)
    sim = CoreSim(nc, trace=os.environ.get("KV6_TRACE", "0") == "1")
    n_st = b_core // ROWS_ST
    xt_host = np.ascontiguousarray(
        x.reshape(128, n_st, 4, G, D_IN).transpose(1, 3, 4, 2, 0)
    ).reshape(n_st, 100, 512).astype(ml_dtypes.bfloat16)
    xt_host = np.concatenate([xt_host, xt_host[:4]])
    sim.tensor("x")[:] = xt_host
    for name, shape, ty in CONST_SPECS:
        sim.tensor(name)[:] = consts[name].astype(
            ml_dtypes.bfloat16 if ty == "bf16" else np.float32)
    sim.simulate(check_with_hw=False)
    out_np = np.asarray(sim.tensor("out"))
    got = _detranspose_out(out_np, n_st, b_core).astype(np.float64)

    ref = reference_np(x.astype(np.float64),
                       {k: v.astype(np.float64) for k, v in w.items()})
    rel = np.linalg.norm(got - ref) / np.linalg.norm(ref)
    mx = np.abs(got - ref).max() / np.abs(ref).max()
    per_pass = sim.time / REPEAT
    per_st = per_pass / (n_st)
    print(f"SIM rel_l2={rel:.3e}  scaled_absmax={mx:.3e}  "
          f"sim_time={sim.time}  per_pass={per_pass:.0f}ns  "
          f"per_st={per_st:.0f}ns")
    assert rel < 2e-2, "simulation mismatch"
    print("SIM OK")
